# revision 63
# baseline (speedup 1.0000x reference)
"""Trainium2 Bass kernel for nn_BasicTransformerBlock (self-attn + cross-attn + GEGLU).

Sharding: data-parallel over the 2048 tokens (256 per core, 8 cores). K/V for
self-attention are computed for the OWN 256 tokens only (LN1 folded into the
projections: W@x with a rank-1 mean correction and an rstd epilogue) and
AllGathered via one mesh collective (~1MB/rank, V padded to 65 feats/head so
the ones-column for softmax denominators ships with it and every gather DMA
is contiguous). Q/K2/V2 projections and phase-B context work run under the
collective; per-rank gather DMAs spread over three queues.

Self-attention runs per head-PAIR: the two heads of a pair occupy partition
halves 0-63/64-127 of the same K tile, so one full-128 stationary load (FWL)
serves both score matmuls against zero-masked Q copies (Q_me/Q_mo). Softmax
denominators come from the V ones-column (row 64 of the AV accumulation);
1/den is a 64-lane DVE reciprocal of a PE-broadcast, multiplied into the
numerators straight out of PSUM. Out-projections accumulate it-outer (two
oc accumulators per PSUM bank) so they pipeline under the attention tail.
GEGLU/ffout weights prefetch into pools that reuse attn1's freed SBUF.
"""

import numpy as np
import ml_dtypes

import concourse.bass as bass
import concourse.mybir as mybir
import concourse.tile as tile
from concourse import bacc
from concourse.bass_utils import run_bass_kernel_spmd

F32 = mybir.dt.float32
F32R = mybir.dt.float32r
BF16 = mybir.dt.bfloat16
AF = mybir.ActivationFunctionType
OP = mybir.AluOpType

P = 128
N, D = 2048, 1024
H, DH = 16, 64
CN, CD = 77, 768
FF = 4096
EPS = 1e-5
SCALE = DH ** -0.5
NCORES = 8
TO = N // NCORES          # 256 tokens owned per core
DT = D // P               # 8 feature tiles
CT = CD // P              # 6 context-feature tiles
NKT = N // P              # 16 key tiles
FT = FF // P              # 32 ffn-inner tiles


def _ln_feature_major(nc, lnp, sbp, consts, src_of, dst_of, n_dt, tn, chunk,
                      post_cb=None):
    """Un-affine LayerNorm over feature-major f32r data."""
    ones_col, ones_row, eps_t = consts
    inv_d = 1.0 / (n_dt * P)
    for tci in range(tn // chunk):
        srcs = [src_of(dt, tci) for dt in range(n_dt)]   # f32r tiles
        sum_ps = lnp.tile([1, chunk], F32, tag="ln_sum", bufs=2)
        for dt in range(n_dt):
            nc.tensor.matmul(sum_ps, ones_col, srcs[dt],
                             start=(dt == 0), stop=(dt == n_dt - 1))
        sumsq_ps = lnp.tile([1, chunk], F32, tag="ln_sumsq", bufs=2)
        for dt in range(n_dt):
            sq_t = sbp.tile([P, chunk], F32R, tag="ln_sq", bufs=2)
            nc.scalar.activation(sq_t, srcs[dt].bitcast(F32), AF.Square)
            nc.tensor.matmul(sumsq_ps, ones_col, sq_t,
                             start=(dt == 0), stop=(dt == n_dt - 1))
        mu_row = sbp.tile([1, chunk], F32R, tag="ln_mu", bufs=2)
        nc.scalar.mul(out=mu_row, in_=sum_ps, mul=inv_d)
        var_row = sbp.tile([1, chunk], F32, tag="ln_var", bufs=2)
        nc.scalar.mul(out=var_row, in_=sumsq_ps, mul=inv_d)
        musq = sbp.tile([1, chunk], F32, tag="ln_musq", bufs=2)
        nc.vector.tensor_mul(out=musq, in0=mu_row.bitcast(F32),
                             in1=mu_row.bitcast(F32))
        nc.vector.tensor_tensor(out=var_row, in0=var_row, in1=musq,
                                op=OP.subtract)
        nc.scalar.activation(var_row, var_row, AF.Sqrt, bias=eps_t)
        rstd_row = sbp.tile([1, chunk], F32R, tag="ln_rstd", bufs=2)
        with nc.allow_low_precision("f32r keeps full fp32 bits here"):
            nc.vector.reciprocal(rstd_row, var_row)
        mu_b = lnp.tile([P, chunk], F32, tag="ln_mub", bufs=1)
        nc.tensor.matmul(mu_b, ones_row, mu_row, start=True, stop=True)
        rstd_b = lnp.tile([P, chunk], F32, tag="ln_rstdb", bufs=1)
        nc.tensor.matmul(rstd_b, ones_row, rstd_row, start=True, stop=True)
        mu_s = sbp.tile([P, chunk], F32, tag="ln_mus", bufs=2)
        nc.scalar.copy(out=mu_s, in_=mu_b)
        for dt in range(n_dt):
            tmp = sbp.tile([P, chunk], F32, tag="ln_tmp", bufs=3)
            eng = nc.gpsimd if dt % 2 else nc.vector
            src_in = mu_s if dt % 2 else mu_b
            eng.tensor_tensor(out=tmp, in0=srcs[dt].bitcast(F32),
                              in1=src_in, op=OP.subtract)
            nc.vector.tensor_tensor(out=dst_of(dt, tci), in0=tmp, in1=rstd_b,
                                    op=OP.mult)
        if post_cb is not None:
            post_cb(tci)


def _ln_stats(nc, lnp, sbp, consts, src_of, n_dt, tn, want_cols=False):
    """Mean/rstd stats for un-affine LN over feature-major f32r data.

    Returns (negmu_row bf16 [1,tn], rstd_b f32 SBUF [P,tn],
             rstd_cols: list of [P,1] f32 SBUF per 128-token chunk or None).
    """
    ones_col, ones_row, eps_t = consts
    inv_d = 1.0 / (n_dt * P)
    srcs = [src_of(dt) for dt in range(n_dt)]
    sum_ps = lnp.tile([1, tn], F32, tag="ln_sum", bufs=1)
    for dt in range(n_dt):
        nc.tensor.matmul(sum_ps, ones_col, srcs[dt],
                         start=(dt == 0), stop=(dt == n_dt - 1))
    sumsq_ps = lnp.tile([1, tn], F32, tag="ln_sumsq", bufs=1)
    for dt in range(n_dt):
        sq_t = sbp.tile([P, tn], F32R, tag="ln_sq", bufs=2)
        nc.scalar.activation(sq_t, srcs[dt].bitcast(F32), AF.Square)
        nc.tensor.matmul(sumsq_ps, ones_col, sq_t,
                         start=(dt == 0), stop=(dt == n_dt - 1))
    mu_row = sbp.tile([1, tn], F32R, tag="ln_mu", bufs=2)
    nc.scalar.mul(out=mu_row, in_=sum_ps, mul=inv_d)
    negmu_row = sbp.tile([1, tn], BF16, tag="ln_nmu", bufs=2)
    nc.scalar.mul(out=negmu_row, in_=sum_ps, mul=-inv_d)
    var_row = sbp.tile([1, tn], F32, tag="ln_var", bufs=2)
    nc.scalar.mul(out=var_row, in_=sumsq_ps, mul=inv_d)
    musq = sbp.tile([1, tn], F32, tag="ln_musq", bufs=2)
    nc.vector.tensor_mul(out=musq, in0=mu_row.bitcast(F32),
                         in1=mu_row.bitcast(F32))
    nc.vector.tensor_tensor(out=var_row, in0=var_row, in1=musq,
                            op=OP.subtract)
    nc.scalar.activation(var_row, var_row, AF.Sqrt, bias=eps_t)
    rstd_row = sbp.tile([1, tn], F32R, tag="ln_rstd", bufs=2)
    with nc.allow_low_precision("f32r keeps full fp32 bits here"):
        nc.vector.reciprocal(rstd_row, var_row)
    rb_ps = lnp.tile([P, tn], F32, tag="ln_rb", bufs=1)
    nc.tensor.matmul(rb_ps, ones_row, rstd_row, start=True, stop=True)
    rstd_b = sbp.tile([P, tn], F32, tag="ln_rbs", bufs=2)
    nc.scalar.copy(out=rstd_b, in_=rb_ps)
    rstd_cols = None
    if want_cols:
        rstd_cols = []
        for kl in range(tn // P):
            rc_ps = lnp.tile([P, 2], F32, tag="ln_rc", bufs=1)
            nc.tensor.matmul(rc_ps, rstd_row[:, kl * P:(kl + 1) * P],
                             ones_row[:, 0:2], start=True, stop=True)
            rc = sbp.tile([P, 1], F32, tag="ln_rcs", bufs=2)
            nc.scalar.copy(out=rc, in_=rc_ps[:, 0:1])
            rstd_cols.append(rc)
    return negmu_row, rstd_b, rstd_cols


def build(flags):
    has_qkv1b, has_bo1, has_q2b, has_bo2, has_gegb, has_ffb = flags
    nc = bacc.Bacc()

    xoT = nc.dram_tensor("xoT", [D, TO], F32R, kind="ExternalInput")
    ctxT = nc.dram_tensor("ctxT", [CD, CN], BF16, kind="ExternalInput")
    wq1T = nc.dram_tensor("wq1T", [D, D], BF16, kind="ExternalInput")
    wk1T = nc.dram_tensor("wk1T", [D, D], BF16, kind="ExternalInput")
    wv1T = nc.dram_tensor("wv1T", [D, D], BF16, kind="ExternalInput")
    wo1T = nc.dram_tensor("wo1T", [D, D], BF16, kind="ExternalInput")
    wq2T = nc.dram_tensor("wq2T", [D, D], BF16, kind="ExternalInput")
    wk2T = nc.dram_tensor("wk2T", [CD, D], BF16, kind="ExternalInput")
    wv2T = nc.dram_tensor("wv2T", [CD, D], BF16, kind="ExternalInput")
    wo2T = nc.dram_tensor("wo2T", [D, D], BF16, kind="ExternalInput")
    wgT = nc.dram_tensor("wgT", [D, 2 * FF], BF16, kind="ExternalInput")
    wfT = nc.dram_tensor("wfT", [FF, D], BF16, kind="ExternalInput")
    onesc = nc.dram_tensor("onesc", [P, 1], F32R, kind="ExternalInput")
    onesr = nc.dram_tensor("onesr", [1, P], F32R, kind="ExternalInput")
    onesb = nc.dram_tensor("onesb", [1, 512], BF16, kind="ExternalInput")
    # row sums of (gain-folded) W, for the folded-LN mean correction
    srows = {nm: nc.dram_tensor(nm, [1, D], BF16, kind="ExternalInput")
             for nm in ("s_q1", "s_k1", "s_v1", "s_q2")}
    selhT = nc.dram_tensor("selh", [H, H * 64], F32R, kind="ExternalInput")
    bias_rows = {}
    bias_cols = {}
    if has_qkv1b:
        for nm in ("bq1", "bk1", "bv1"):
            bias_rows[nm] = nc.dram_tensor(nm, [1, D], BF16, kind="ExternalInput")
        for nm in ("bq1c", "bk1c"):
            bias_cols[nm] = nc.dram_tensor(nm, [P, DT], BF16,
                                           kind="ExternalInput")
    if has_bo1:
        bias_rows["bo1"] = nc.dram_tensor("bo1", [1, D], BF16, kind="ExternalInput")
    if has_q2b:
        bias_rows["bq2"] = nc.dram_tensor("bq2", [1, D], BF16, kind="ExternalInput")
        bias_cols["bq2c"] = nc.dram_tensor("bq2c", [P, DT], BF16,
                                           kind="ExternalInput")
    if has_bo2:
        bias_rows["bo2"] = nc.dram_tensor("bo2", [1, D], BF16, kind="ExternalInput")
    if has_gegb:
        bias_rows["bgeg"] = nc.dram_tensor("bgeg", [1, 2 * FF], BF16,
                                           kind="ExternalInput")
    if has_ffb:
        bias_rows["bff"] = nc.dram_tensor("bff", [1, D], BF16, kind="ExternalInput")
    yT = nc.dram_tensor("yT", [D, TO], F32R, kind="ExternalOutput")

    xoT_v = xoT.rearrange("(dt p) t -> dt p t", p=P)
    ctxT_v = ctxT.rearrange("(ct p) t -> ct p t", p=P)
    yT_v = yT.rearrange("(dt p) t -> p dt t", p=P)

    def wview(w):
        return w.rearrange("(it p) o -> p it o", p=P)

    with tile.TileContext(nc) as tc:
        with tc.tile_pool(name="consts", bufs=1) as cpool, \
             tc.tile_pool(name="pers", bufs=1) as pers, \
             tc.tile_pool(name="wmain", bufs=1) as wmain:

            ones_col = cpool.tile([P, 1], F32R)
            nc.scalar.dma_start(ones_col, onesc[:])
            ones_row = cpool.tile([1, P], F32R)
            nc.scalar.dma_start(ones_row, onesr[:])
            ones_b = cpool.tile([1, 512], BF16)
            nc.scalar.dma_start(ones_b, onesb[:])
            eps_t = cpool.tile([1, 1], F32)
            nc.vector.memset(eps_t, EPS)
            consts = (ones_col, ones_row, eps_t)

            bias_sb = {}
            for nm, t in bias_rows.items():
                bt = cpool.tile([1, t.shape[1]], BF16, tag=f"bias_{nm}")
                nc.scalar.dma_start(bt, t[:])
                bias_sb[nm] = bt
            for nm, t in bias_cols.items():
                bt = cpool.tile([P, DT], BF16, tag=f"bias_{nm}")
                nc.scalar.dma_start(bt, t[:])
                bias_sb[nm] = bt
            srow_sb = {}
            for nm, t in srows.items():
                st = cpool.tile([1, D], BF16, tag=f"srow_{nm}")
                nc.scalar.dma_start(st, t[:])
                srow_sb[nm] = st
            sq1_sb, sk1_sb, sv1_sb, sq2_sb = (
                srow_sb["s_q1"], srow_sb["s_k1"], srow_sb["s_v1"],
                srow_sb["s_q2"])
            selh_sb = cpool.tile([H, H * 64], F32R)
            nc.scalar.dma_start(selh_sb, selhT[:])
            bq1c_sb = bias_sb.get("bq1c")
            bk1c_sb = bias_sb.get("bk1c")
            bq2c_sb = bias_sb.get("bq2c")

            def proj_it_outer(pp, w_sb, act, out_cb, bias=None, tag="po"):
                """Out-projection with it as the outer loop: each input tile's
                contribution lands in all 8 oc accumulators, so the matmuls
                pipeline as the producer of act[:, it, :] finishes (two ocs
                share a PSUM bank via disjoint halves)."""
                banks = [pp.tile([P, 2, TO], F32, tag=f"{tag}{j}",
                                 name=f"{tag}_bank{j}", bufs=1)
                         for j in range(DT // 2)]
                for it in range(DT):
                    for oc in range(DT):
                        nc.tensor.matmul(
                            banks[oc // 2][:, oc % 2, :],
                            w_sb[:, it, oc * P:(oc + 1) * P],
                            act[:, it, :],
                            start=(it == 0 and oc % 2 == 0),
                            stop=(it == DT - 1 and bias is None),
                            skip_group_check=(oc % 2 == 1))
                for oc in range(DT):
                    if bias is not None:
                        nc.tensor.matmul(
                            banks[oc // 2][:, oc % 2, :],
                            bias[:, oc * P:(oc + 1) * P],
                            ones_b[:, :TO], start=False, stop=True,
                            skip_group_check=(oc % 2 == 1))
                    out_cb(oc, banks[oc // 2][:, oc % 2, :])

            def proj_feature_major(pp, w_sb, act, out_cb, n_in, n_tok,
                                   bias=None, tag="pp256"):
                """out[oc] = sum_it w.T @ act; out_cb(oc, psum)."""
                for oc in range(DT):
                    ps = pp.tile([P, n_tok], F32, tag=tag, bufs=2)
                    for it in range(n_in):
                        nc.tensor.matmul(ps, w_sb[:, it, oc * P:(oc + 1) * P],
                                         act[:, it, :],
                                         start=(it == 0),
                                         stop=(it == n_in - 1 and bias is None))
                    if bias is not None:
                        nc.tensor.matmul(ps, bias[:, oc * P:(oc + 1) * P],
                                         ones_b[:, :n_tok], start=False,
                                         stop=True)
                    out_cb(oc, ps)

            x_ownT = pers.tile([P, DT, TO], F32R)      # residual stream (own)
            nc.sync.dma_start(x_ownT[:, :, :],
                              xoT.rearrange("(dt p) t -> p dt t", p=P))

            # cross-attn K2/V2 depend only on the context: computed early in
            # phase B so they overlap everything up to phase E.
            K2_sb = pers.tile([P, DT, CN], BF16)
            V2_sb = pers.tile([P, H, 65], BF16)

            # ========== attn1 scope: phases A-D ==========
            with tc.tile_pool(name="c1", bufs=1) as c1:
                O_sb = c1.tile([P, DT, TO], BF16)
                K_sb = c1.tile([P, DT, N], BF16)
                V_sb = c1.tile([P, NKT, H, 65], BF16)
                # Q with the other head of each pair zeroed: lets scores use
                # the full-128 K stationary (one FWL load per key tile for
                # both heads; the masked rows contribute exactly zero)
                Q_me = c1.tile([P, DT, TO], BF16)
                Q_mo = c1.tile([P, DT, TO], BF16)
                xb1 = c1.tile([P, DT, TO], BF16)
                nc.vector.memset(Q_me[64:128, :, :], 0.0)
                nc.gpsimd.memset(Q_mo[0:64, :, :], 0.0)

                # ----- Phase A: K/V own (LN1 folded) -> AllGather; Q under AG -
                scopeA = nc.enter_named_scope("phA_ln1", False)
                # weight prefetches first: DMA runs under stats/cast compute
                wk1_sb = wmain.tile([P, DT, D], BF16, tag="w2m", bufs=2)
                nc.scalar.dma_start(wk1_sb, wview(wk1T))
                wv1_sb = wmain.tile([P, DT, D], BF16, tag="w2m", bufs=2)
                nc.sync.dma_start(wv1_sb, wview(wv1T))
                wq1_sb = wmain.tile([P, DT, D], BF16, tag="w2m", bufs=2)
                nc.gpsimd.dma_start(wq1_sb, wview(wq1T))
                # phase-B inputs prefetched NOW so their DMAs run under phA
                # compute and the K2/V2 matmuls fit under the collective
                wbpool = tc.tile_pool(name="wb", bufs=1)
                wpool = wbpool.__enter__()
                ctx_sb = wpool.tile([P, CT, CN], BF16, tag="ctx", bufs=1)
                for ct in range(CT):
                    nc.scalar.dma_start(ctx_sb[:, ct, :], ctxT_v[ct])
                wk2_sb = wpool.tile([P, CT, D], BF16, tag="w15", bufs=2)
                nc.scalar.dma_start(wk2_sb, wview(wk2T))
                wv2_sb = wpool.tile([P, CT, D], BF16, tag="w15", bufs=2)
                nc.scalar.dma_start(wv2_sb, wview(wv2T))
                bk1 = bias_sb.get("bk1")
                bv1 = bias_sb.get("bv1")
                H65 = H * 65            # V padded to 65 feats/head (ones col)
                KSZ = DT * P * TO            # 262144 elems
                VSZ = 2 * P * H65            # 266240 elems
                BSZ = KSZ + VSZ
                with tc.tile_pool(name="dramb", bufs=1, space="DRAM") as dram:
                    cb_in = dram.tile([BSZ, 1], BF16)
                    cb_out = dram.tile([NCORES * BSZ, 1], BF16,
                                       addr_space="Shared")
                    cbin_k = cb_in[0:KSZ, :].rearrange(
                        "(dt p t) o -> p dt (t o)", dt=DT, p=P, t=TO)
                    cbin_v = cb_in[KSZ:BSZ, :].rearrange(
                        "(kl p f) o -> p kl (f o)", kl=2, p=P, f=H65)
                    cbo_k = [cb_out[r * BSZ:r * BSZ + KSZ, :].rearrange(
                        "(dt p t) o -> p dt (t o)", dt=DT, p=P, t=TO)
                        for r in range(NCORES)]
                    cbo_v = [cb_out[r * BSZ + KSZ:(r + 1) * BSZ, :].rearrange(
                        "(kl p f) o -> p kl (f o)", kl=2, p=P, f=H65)
                        for r in range(NCORES)]

                    with tc.tile_pool(name="lnps", bufs=2, space="PSUM") as lnp, \
                         tc.tile_pool(name="lnsb", bufs=2) as lnsb, \
                         tc.tile_pool(name="kvsb", bufs=1) as kvsb, \
                         tc.tile_pool(name="projps", bufs=2, space="PSUM") as pp:
                        for dt in range(DT):
                            if dt % 2:
                                nc.vector.tensor_copy(
                                    out=xb1[:, dt, :],
                                    in_=x_ownT[:, dt, :].bitcast(F32))
                            else:
                                nc.scalar.copy(
                                    out=xb1[:, dt, :],
                                    in_=x_ownT[:, dt, :].bitcast(F32))
                        negmu1, rstd1_b, rstd1_c = _ln_stats(
                            nc, lnp, lnsb, consts,
                            lambda dt: x_ownT[:, dt, :], DT, TO,
                            want_cols=True)
                        Kown = kvsb.tile([P, DT, TO], BF16)
                        Vown = kvsb.tile([P, 2, H, 65], BF16)
                        nc.vector.memset(Vown[:, :, :, DH:65], 1.0)
                        for oc in range(DT):
                            k_ps = pp.tile([P, TO], F32, tag="pp256", bufs=2)
                            for it in range(DT):
                                nc.tensor.matmul(
                                    k_ps, wk1_sb[:, it, oc * P:(oc + 1) * P],
                                    xb1[:, it, :],
                                    start=(it == 0), stop=False)
                            nc.tensor.matmul(
                                k_ps, sk1_sb[:, oc * P:(oc + 1) * P],
                                negmu1, start=False, stop=True)
                            if oc % 2:
                                tmp = lnsb.tile([P, TO], F32, tag="ep_t",
                                                bufs=3)
                                nc.scalar.copy(out=tmp, in_=k_ps)
                                nc.gpsimd.tensor_tensor(
                                    out=Kown[:, oc, :], in0=tmp,
                                    in1=rstd1_b, op=OP.mult)
                            else:
                                nc.vector.tensor_tensor(
                                    out=Kown[:, oc, :], in0=k_ps,
                                    in1=rstd1_b, op=OP.mult)
                            if bk1 is not None:
                                nc.vector.tensor_scalar_add(
                                    out=Kown[:, oc, :], in0=Kown[:, oc, :],
                                    scalar1=bk1c_sb[:, oc:oc + 1])
                        for kl in range(2):
                            for hc in range(2):
                                v_ps = pp.tile([P, 512], F32, tag="pp512",
                                               bufs=2)
                                for it in range(DT):
                                    nc.tensor.matmul(
                                        v_ps,
                                        xb1[:, it, kl * P:(kl + 1) * P],
                                        wv1_sb[:, it, hc * 512:(hc + 1) * 512],
                                        start=(it == 0), stop=False)
                                nc.tensor.matmul(
                                    v_ps, negmu1[:, kl * P:(kl + 1) * P],
                                    sv1_sb[:, hc * 512:(hc + 1) * 512],
                                    start=False, stop=(bv1 is None))
                                if bv1 is not None:
                                    nc.tensor.matmul(
                                        v_ps, ones_row.bitcast(BF16),
                                        bv1[:, hc * 512:(hc + 1) * 512],
                                        start=False, stop=True)
                                nc.scalar.activation(
                                    Vown[:, kl, hc * 8:(hc + 1) * 8, 0:DH],
                                    v_ps.rearrange("p (h d) -> p h d", d=DH),
                                    AF.Copy, scale=rstd1_c[kl])
                        nc.gpsimd.dma_start(cbin_k, Kown[:, :, :])
                        nc.scalar.dma_start(cbin_v, Vown[:, :, :, :])
                        nc.gpsimd.collective_compute(
                            "AllGather", mybir.AluOpType.bypass,
                            replica_groups=[list(range(NCORES))],
                            ins=[cb_in[:, :].opt()],
                            outs=[cb_out[:, :].opt()])

                        # Q projection: independent of the AG, overlaps it
                        for oc in range(DT):
                            q_ps = pp.tile([P, TO], F32, tag="pp256", bufs=2)
                            for it in range(DT):
                                nc.tensor.matmul(
                                    q_ps, wq1_sb[:, it, oc * P:(oc + 1) * P],
                                    xb1[:, it, :],
                                    start=(it == 0), stop=False)
                            nc.tensor.matmul(
                                q_ps, sq1_sb[:, oc * P:(oc + 1) * P],
                                negmu1, start=False, stop=True)
                            nc.vector.tensor_tensor(
                                out=Q_me[0:64, oc, :], in0=q_ps[0:64, :],
                                in1=rstd1_b[0:64, :], op=OP.mult)
                            nc.vector.tensor_tensor(
                                out=Q_mo[64:128, oc, :], in0=q_ps[64:128, :],
                                in1=rstd1_b[64:128, :], op=OP.mult)
                            if bias_sb.get("bq1") is not None:
                                nc.vector.tensor_scalar_add(
                                    out=Q_me[0:64, oc, :],
                                    in0=Q_me[0:64, oc, :],
                                    scalar1=bq1c_sb[0:64, oc:oc + 1])
                                nc.vector.tensor_scalar_add(
                                    out=Q_mo[64:128, oc, :],
                                    in0=Q_mo[64:128, oc, :],
                                    scalar1=bq1c_sb[64:128, oc:oc + 1])
                    nc.leave_named_scope("phA_ln1", scopeA[0], False)

                    # ----- Phase B: K2/V2 (context), also under the AG -----
                    scopeB = nc.enter_named_scope("phB_qkv", False)
                    with tc.tile_pool(name="projps2", bufs=2, space="PSUM") as pp:
                        for oc in range(DT):
                            k_ps = pp.tile([P, CN], F32, tag="ppsm", bufs=2)
                            for it in range(CT):
                                nc.tensor.matmul(
                                    k_ps, wk2_sb[:, it, oc * P:(oc + 1) * P],
                                    ctx_sb[:, it, :],
                                    start=(it == 0), stop=(it == CT - 1))
                            nc.scalar.copy(out=K2_sb[:, oc, :], in_=k_ps)
                        nc.vector.memset(V2_sb[:, :, DH:65], 1.0)
                        for hc in range(2):
                            v_ps = pp.tile([CN, 512], F32, tag="ppsm", bufs=2)
                            for it in range(CT):
                                nc.tensor.matmul(
                                    v_ps, ctx_sb[:, it, :],
                                    wv2_sb[:, it, hc * 512:(hc + 1) * 512],
                                    start=(it == 0), stop=(it == CT - 1))
                            nc.scalar.copy(
                                out=V2_sb[0:CN, hc * 8:(hc + 1) * 8, 0:64],
                                in_=v_ps.rearrange("p (h d) -> p h d", d=64))
                    wbpool.__exit__(None, None, None)
                    nc.leave_named_scope("phB_qkv", scopeB[0], False)

                    # gather-in: K/V for all 2048 tokens from the collectives,
                    # contiguous per-rank DMAs spread over three queues
                    gq = [nc.sync, nc.scalar, nc.gpsimd]
                    for r in range(NCORES):
                        gq[r % 3].dma_start(
                            K_sb[:, :, r * TO:(r + 1) * TO], cbo_k[r])
                        gq[(r + 1) % 3].dma_start(
                            V_sb[:, 2 * r:2 * r + 2, :, :], cbo_v[r])

                # ----- Phase C: self-attention heads -----
                # Denominators are deferred: each head's numerator goes to
                # SBUF (f32), its denominator row is DMAd into den_all; one
                # 16-lane reciprocal then feeds per-head PE broadcasts.
                scopeC = nc.enter_named_scope("phC_attn", False)
                with tc.tile_pool(name="aps", bufs=1, space="PSUM") as apsum, \
                     tc.tile_pool(name="asb", bufs=1) as asb:
                    for j in range(H // 2):       # head pair (2j, 2j+1)
                        o_pair = apsum.tile([65, 2, TO], F32, tag="o_pair",
                                            bufs=2)
                        e_store = {}
                        for kt2 in range(NKT // 2 + 1):
                            if kt2 < NKT // 2:
                                s_e = apsum.tile([P, 2, TO], F32,
                                                 tag="s_e", bufs=2)
                                s_o = apsum.tile([P, 2, TO], F32,
                                                 tag="s_o", bufs=2)
                                for half in range(2):
                                    kt = kt2 * 2 + half
                                    ks = K_sb[:, j, kt * P:(kt + 1) * P]
                                    nc.tensor.matmul(
                                        s_e[:, half, :], ks, Q_me[:, j, :],
                                        start=(half == 0), stop=True,
                                        skip_group_check=(half == 1))
                                    nc.tensor.matmul(
                                        s_o[:, half, :], ks, Q_mo[:, j, :],
                                        start=(half == 0), stop=True,
                                        skip_group_check=(half == 1))
                                e_e = asb.tile([P, 2, TO], BF16, tag="e_t",
                                               bufs=8)
                                nc.scalar.activation(e_e, s_e, AF.Exp,
                                                     scale=SCALE)
                                e_o = asb.tile([P, 2, TO], BF16, tag="e_t",
                                               bufs=8)
                                nc.scalar.activation(e_o, s_o, AF.Exp,
                                                     scale=SCALE)
                                e_store[kt2] = (e_e, e_o)
                            if kt2 > 0:
                                e_e, e_o = e_store.pop(kt2 - 1)
                                for half in range(2):
                                    kt = (kt2 - 1) * 2 + half
                                    nc.tensor.matmul(
                                        o_pair[:, 0, :],
                                        V_sb[:, kt, 2 * j, :],
                                        e_e[:, half, :],
                                        start=(kt == 0),
                                        stop=(kt == NKT - 1))
                                    nc.tensor.matmul(
                                        o_pair[:, 1, :],
                                        V_sb[:, kt, 2 * j + 1, :],
                                        e_o[:, half, :],
                                        start=False,
                                        stop=(kt == NKT - 1),
                                        skip_group_check=True)
                        # divide: den -> SBUF, PE broadcast to 64 rows,
                        # 64-lane DVE reciprocal, multiply (no ACT tables)
                        den_sb = asb.tile([1, 2, TO], F32R, tag="den", bufs=2)
                        nc.scalar.copy(out=den_sb, in_=o_pair[64:65, :, :])
                        r_ps = apsum.tile([64, 2, TO], F32, tag="r_ps",
                                          bufs=2)
                        for hh in range(2):
                            nc.tensor.matmul(r_ps[:, hh, :],
                                             ones_row[:, :64], den_sb[:, hh, :],
                                             start=(hh == 0), stop=True,
                                             skip_group_check=(hh == 1))
                        r_bc = asb.tile([64, 2, TO], F32R, tag="r_bc", bufs=2)
                        with nc.allow_low_precision("f32r == f32 bits"):
                            nc.vector.reciprocal(r_bc, r_ps)
                        for hh in range(2):
                            nc.vector.tensor_tensor(
                                out=O_sb[hh * 64:hh * 64 + 64, j, :],
                                in0=o_pair[0:64, hh, :],
                                in1=r_bc[:, hh, :].bitcast(F32),
                                op=OP.mult)
                nc.leave_named_scope("phC_attn", scopeC[0], False)

                # ----- Phase D: attn1 out-proj + residual -----
                scopeD = nc.enter_named_scope("phD_oproj", False)
                with tc.tile_pool(name="dps", bufs=1, space="PSUM") as pp:
                    wo1_sb = wmain.tile([P, DT, D], BF16, tag="w2m", bufs=2)
                    nc.sync.dma_start(wo1_sb, wview(wo1T))

                    def add_residual(oc, ps):
                        nc.vector.tensor_tensor(
                            out=x_ownT[:, oc, :],
                            in0=x_ownT[:, oc, :].bitcast(F32),
                            in1=ps, op=OP.add)

                    proj_it_outer(pp, wo1_sb, O_sb, add_residual,
                                  bias=bias_sb.get("bo1"), tag="dps")
                nc.leave_named_scope("phD_oproj", scopeD[0], False)

            # GEGLU weight pool opens here: it reuses attn1's just-freed
            # SBUF region, so the first two chunk pairs stream in during
            # phase E instead of stalling the phF pipeline start
            wgT_v = wview(wgT)
            wgpool_cm = tc.tile_pool(name="wg", bufs=1)
            wgpool = wgpool_cm.__enter__()
            wg_pre = []
            for g in range(2):
                wh_p = wgpool.tile([P, DT, 512], BF16, tag="wgh",
                                   name=f"wgh_pre{g}", bufs=2)
                nc.gpsimd.dma_start(wh_p, wgT_v[:, :, g * 512:(g + 1) * 512])
                wg_p = wgpool.tile([P, DT, 512], BF16, tag="wgg",
                                   name=f"wgg_pre{g}", bufs=2)
                nc.gpsimd.dma_start(
                    wg_p, wgT_v[:, :, FF + g * 512:FF + (g + 1) * 512])
                wg_pre.append((wh_p, wg_p))
            wfT_pre_v = wfT.rearrange("(f p) o -> f p o", p=P)
            wfpool_cm = tc.tile_pool(name="wfp", bufs=1)
            wfpool = wfpool_cm.__enter__()
            wf_pre = []
            wfq0 = [nc.scalar, nc.gpsimd]
            for f8 in range(12):
                wt = wfpool.tile([P, D], BF16, tag="wft",
                                 name=f"wft_pre{f8}", bufs=12)
                wfq0[f8 % 2].dma_start(wt, wfT_pre_v[f8])
                wf_pre.append(wt)

            # ========== attn2 scope: phase E ==========
            scopeE = nc.enter_named_scope("phE_xattn", False)
            with tc.tile_pool(name="ce", bufs=1) as ce:
                xb2 = ce.tile([P, DT, TO], BF16)
                Q2_sb = ce.tile([P, DT, TO], BF16)
                O2_sb = ce.tile([P, DT, TO], BF16)

                # LN2 folded into the Q2 projection (like LN1 in phase A)
                with tc.tile_pool(name="lnps2", bufs=2, space="PSUM") as lnp, \
                     tc.tile_pool(name="lnsb2", bufs=2) as lnsb, \
                     tc.tile_pool(name="eps_", bufs=2, space="PSUM") as pp:
                    wq2_sb = wmain.tile([P, DT, D], BF16, tag="w2m", bufs=2)
                    nc.sync.dma_start(wq2_sb, wview(wq2T))
                    for dt in range(DT):
                        if dt % 2:
                            nc.vector.tensor_copy(
                                out=xb2[:, dt, :],
                                in_=x_ownT[:, dt, :].bitcast(F32))
                        else:
                            nc.scalar.copy(
                                out=xb2[:, dt, :],
                                in_=x_ownT[:, dt, :].bitcast(F32))
                    negmu2, rstd2_b, _ = _ln_stats(
                        nc, lnp, lnsb, consts,
                        lambda dt: x_ownT[:, dt, :], DT, TO)
                    for oc in range(DT):
                        q_ps = pp.tile([P, TO], F32, tag="pp256", bufs=2)
                        for it in range(DT):
                            nc.tensor.matmul(
                                q_ps, wq2_sb[:, it, oc * P:(oc + 1) * P],
                                xb2[:, it, :],
                                start=(it == 0), stop=False)
                        nc.tensor.matmul(
                            q_ps, sq2_sb[:, oc * P:(oc + 1) * P],
                            negmu2, start=False, stop=True)
                        if oc % 2:
                            tmp = lnsb.tile([P, TO], F32, tag="ep_t", bufs=3)
                            nc.scalar.copy(out=tmp, in_=q_ps)
                            nc.gpsimd.tensor_tensor(
                                out=Q2_sb[:, oc, :], in0=tmp,
                                in1=rstd2_b, op=OP.mult)
                        else:
                            nc.vector.tensor_tensor(
                                out=Q2_sb[:, oc, :], in0=q_ps,
                                in1=rstd2_b, op=OP.mult)
                        if bq2c_sb is not None:
                            nc.vector.tensor_scalar_add(
                                out=Q2_sb[:, oc, :], in0=Q2_sb[:, oc, :],
                                scalar1=bq2c_sb[:, oc:oc + 1])

                with tc.tile_pool(name="aps2", bufs=1, space="PSUM") as apsum, \
                     tc.tile_pool(name="asb2", bufs=1) as asb:
                    for j in range(H // 2):
                        o_l, den_l = [], []
                        for hh in range(2):
                            s_ps = apsum.tile([CN, TO], F32, tag="s_ps",
                                              name=f"s2_{j}_{hh}", bufs=3)
                            nc.tensor.matmul(
                                s_ps, K2_sb[hh * 64:hh * 64 + 64, j, :],
                                Q2_sb[hh * 64:hh * 64 + 64, j, :],
                                start=True, stop=True)
                            e_t = asb.tile([CN, TO], BF16, tag="e_t",
                                           name=f"e2_{j}_{hh}", bufs=4)
                            nc.scalar.activation(e_t, s_ps, AF.Exp,
                                                 scale=SCALE)
                            o_ps = apsum.tile([65, TO], F32, tag="o_ps",
                                              name=f"o2_{j}_{hh}", bufs=3)
                            nc.tensor.matmul(o_ps, V2_sb[0:CN, 2 * j + hh, :],
                                             e_t, start=True, stop=True)
                            o_l.append(o_ps)
                            den_sb = asb.tile([1, TO], F32R, tag="den",
                                              name=f"d2_{j}_{hh}", bufs=4)
                            nc.scalar.copy(out=den_sb, in_=o_ps[64:65, :])
                            den_l.append(den_sb)
                        r_ps = apsum.tile([64, 2, TO], F32, tag="r_ps",
                                          bufs=2)
                        for hh in range(2):
                            nc.tensor.matmul(r_ps[:, hh, :],
                                             ones_row[:, :64], den_l[hh],
                                             start=(hh == 0), stop=True,
                                             skip_group_check=(hh == 1))
                        r_bc = asb.tile([64, 2, TO], F32R, tag="r_bc", bufs=2)
                        with nc.allow_low_precision("f32r == f32 bits"):
                            nc.vector.reciprocal(r_bc, r_ps)
                        for hh in range(2):
                            nc.vector.tensor_tensor(
                                out=O2_sb[hh * 64:hh * 64 + 64, j, :],
                                in0=o_l[hh][0:64, :],
                                in1=r_bc[:, hh, :].bitcast(F32),
                                op=OP.mult)

                with tc.tile_pool(name="eps2", bufs=1, space="PSUM") as pp:
                    wo2_sb = wmain.tile([P, DT, D], BF16, tag="w2m", bufs=2)
                    nc.sync.dma_start(wo2_sb, wview(wo2T))

                    def add_residual2(oc, ps):
                        nc.vector.tensor_tensor(
                            out=x_ownT[:, oc, :],
                            in0=x_ownT[:, oc, :].bitcast(F32),
                            in1=ps, op=OP.add)

                    proj_it_outer(pp, wo2_sb, O2_sb, add_residual2,
                                  bias=bias_sb.get("bo2"), tag="eps2")
            nc.leave_named_scope("phE_xattn", scopeE[0], False)

            # ========== FFN scope: phase F ==========
            scopeF = nc.enter_named_scope("phF_ffn", False)
            with tc.tile_pool(name="cf", bufs=1) as cf:
                ln3T = cf.tile([P, DT, TO], BF16)
                Hbuf = cf.tile([P, FT, TO], BF16)

                with tc.tile_pool(name="lnps3", bufs=2, space="PSUM") as lnp, \
                     tc.tile_pool(name="lnsb3", bufs=2) as lnsb:
                    _ln_feature_major(
                        nc, lnp, lnsb, consts,
                        lambda dt, tci: x_ownT[:, dt, :],
                        lambda dt, tci: ln3T[:, dt, :],
                        DT, TO, TO)

                bgeg = bias_sb.get("bgeg")
                with tc.tile_pool(name="gps", bufs=1, space="PSUM") as gpsum, \
                     tc.tile_pool(name="gsb", bufs=3) as gsb:
                    for g in range(8):
                        if g < 2:
                            wg_h, wg_g = wg_pre[g]
                        else:
                            wg_h = wgpool.tile([P, DT, 512], BF16, tag="wgh",
                                               bufs=2)
                            nc.sync.dma_start(
                                wg_h, wgT_v[:, :, g * 512:(g + 1) * 512])
                            wg_g = wgpool.tile([P, DT, 512], BF16, tag="wgg",
                                               bufs=2)
                            nc.sync.dma_start(
                                wg_g,
                                wgT_v[:, :, FF + g * 512:FF + (g + 1) * 512])
                        for fi in range(4):
                            f = g * 4 + fi
                            h_ps = gpsum.tile([P, TO], F32, tag="h_ps", bufs=2)
                            for it in range(DT):
                                nc.tensor.matmul(
                                    h_ps, wg_h[:, it, fi * P:(fi + 1) * P],
                                    ln3T[:, it, :],
                                    start=(it == 0),
                                    stop=(it == DT - 1 and bgeg is None))
                            if bgeg is not None:
                                nc.tensor.matmul(
                                    h_ps, bgeg[:, f * P:(f + 1) * P],
                                    ones_b[:, :TO], start=False, stop=True)
                            g_ps = gpsum.tile([P, TO], F32, tag="g_ps", bufs=2)
                            for it in range(DT):
                                nc.tensor.matmul(
                                    g_ps, wg_g[:, it, fi * P:(fi + 1) * P],
                                    ln3T[:, it, :],
                                    start=(it == 0),
                                    stop=(it == DT - 1 and bgeg is None))
                            if bgeg is not None:
                                nc.tensor.matmul(
                                    g_ps,
                                    bgeg[:, FF + f * P:FF + (f + 1) * P],
                                    ones_b[:, :TO], start=False, stop=True)
                            gel = gsb.tile([P, TO], F32, tag="gel", bufs=3)
                            nc.scalar.activation(gel, g_ps, AF.Gelu)
                            nc.vector.tensor_tensor(out=Hbuf[:, f, :],
                                                    in0=h_ps, in1=gel,
                                                    op=OP.mult)

                # ffout: two-level accumulation; spills add into x_ownT
                wfT_v = wfT.rearrange("(f p) o -> f p o", p=P)
                bff = bias_sb.get("bff")
                with tc.tile_pool(name="yps", bufs=2, space="PSUM") as yp_:
                    wfq = [nc.sync, nc.scalar, nc.gpsimd]
                    for fg in range(4):
                        wf_tiles = []
                        for f8 in range(8):
                            fidx = fg * 8 + f8
                            if fidx < 12:
                                wf_tiles.append(wf_pre[fidx])
                                continue
                            wt = wfpool.tile([P, D], BF16, tag="wft", bufs=12)
                            wfq[f8 % 3].dma_start(wt, wfT_v[fidx])
                            wf_tiles.append(wt)
                        for oc in range(DT):
                            i_ps = yp_.tile([P, TO], F32, tag="i_ps")
                            add_bias = bff is not None and fg == 3
                            for f8 in range(8):
                                nc.tensor.matmul(
                                    i_ps, wf_tiles[f8][:, oc * P:(oc + 1) * P],
                                    Hbuf[:, fg * 8 + f8, :],
                                    start=(f8 == 0),
                                    stop=(f8 == 7 and not add_bias))
                            if add_bias:
                                nc.tensor.matmul(
                                    i_ps, bff[:, oc * P:(oc + 1) * P],
                                    ones_b[:, :TO], start=False, stop=True)
                            nc.vector.tensor_tensor(
                                out=x_ownT[:, oc, :],
                                in0=x_ownT[:, oc, :].bitcast(F32),
                                in1=i_ps, op=OP.add)
                            if fg == 3:
                                nc.sync.dma_start(yT_v[:, oc, :],
                                                  x_ownT[:, oc, :])
            wfpool_cm.__exit__(None, None, None)
            wgpool_cm.__exit__(None, None, None)
            nc.leave_named_scope("phF_ffn", scopeF[0], False)

    nc.finalize()
    return nc


_CACHE = {}


def kernel(**inputs):
    def f32c(a):
        return np.ascontiguousarray(np.asarray(a, dtype=np.float32))

    def bfT(w):
        """W [out,in] (optionally gain-folded) -> bf16 W.T contiguous."""
        return np.ascontiguousarray(w.T).astype(ml_dtypes.bfloat16)

    x = f32c(inputs["hidden_states"])[0]          # [N, D]
    ctx = f32c(inputs["context"])[0]              # [CN, CD]
    g1 = f32c(inputs["ln1_g"]); b1 = f32c(inputs["ln1_b"])
    g2 = f32c(inputs["ln2_g"]); b2 = f32c(inputs["ln2_b"])
    g3 = f32c(inputs["ln3_g"]); b3 = f32c(inputs["ln3_b"])
    wq1 = f32c(inputs["wq1"]); wk1 = f32c(inputs["wk1"]); wv1 = f32c(inputs["wv1"])
    wo1 = f32c(inputs["wo1"]); bo1 = f32c(inputs["bo1"])
    wq2 = f32c(inputs["wq2"]); wk2 = f32c(inputs["wk2"]); wv2 = f32c(inputs["wv2"])
    wo2 = f32c(inputs["wo2"]); bo2 = f32c(inputs["bo2"])
    wg = f32c(inputs["w_geglu"]); bg = f32c(inputs["b_geglu"])
    wf = f32c(inputs["w_ffout"]); bf = f32c(inputs["b_ffout"])

    bq1 = wq1 @ b1; bk1 = wk1 @ b1; bv1 = wv1 @ b1
    bq2 = wq2 @ b2
    bgeg = bg + wg @ b3
    flags = (bool(np.any(bq1) or np.any(bk1) or np.any(bv1)), bool(np.any(bo1)),
             bool(np.any(bq2)), bool(np.any(bo2)), bool(np.any(bgeg)),
             bool(np.any(bf)))

    if flags not in _CACHE:
        _CACHE[flags] = build(flags)
    nc = _CACHE[flags]

    xT = np.ascontiguousarray(x.T)                # [D, N]
    bf16 = ml_dtypes.bfloat16
    shared = {
        "ctxT": np.ascontiguousarray(ctx.T).astype(bf16),
        "wq1T": bfT(wq1 * g1[None, :]),
        "wk1T": bfT(wk1 * g1[None, :]),
        "wv1T": bfT(wv1 * g1[None, :]),
        "wo1T": bfT(wo1),
        "wq2T": bfT(wq2 * g2[None, :]),
        "wk2T": bfT(wk2),
        "wv2T": bfT(wv2),
        "wo2T": bfT(wo2),
        "wgT": bfT(wg * g3[None, :]),
        "wfT": bfT(wf),
        "onesc": np.ones((P, 1), np.float32),
        "onesr": np.ones((1, P), np.float32),
        "onesb": np.ones((1, 512), bf16),
        "s_q1": (wq1 * g1[None, :]).sum(1)[None, :].astype(bf16),
        "s_k1": (wk1 * g1[None, :]).sum(1)[None, :].astype(bf16),
        "s_v1": (wv1 * g1[None, :]).sum(1)[None, :].astype(bf16),
        "s_q2": (wq2 * g2[None, :]).sum(1)[None, :].astype(bf16),
        "selh": np.kron(np.eye(16, dtype=np.float32),
                        np.ones((1, 64), np.float32)),
    }

    def colform(b):
        return np.ascontiguousarray(
            b.reshape(D // P, P).T).astype(bf16)

    if flags[0]:
        shared["bq1"] = bq1[None, :].astype(bf16)
        shared["bk1"] = bk1[None, :].astype(bf16)
        shared["bv1"] = bv1[None, :].astype(bf16)
        shared["bq1c"] = colform(bq1)
        shared["bk1c"] = colform(bk1)
    if flags[1]:
        shared["bo1"] = bo1[None, :].astype(bf16)
    if flags[2]:
        shared["bq2"] = bq2[None, :].astype(bf16)
        shared["bq2c"] = colform(bq2)
    if flags[3]:
        shared["bo2"] = bo2[None, :].astype(bf16)
    if flags[4]:
        shared["bgeg"] = bgeg[None, :].astype(bf16)
    if flags[5]:
        shared["bff"] = bf[None, :].astype(bf16)

    in_maps = []
    for c in range(NCORES):
        m = dict(shared)
        m["xoT"] = np.ascontiguousarray(xT[:, c * TO:(c + 1) * TO])
        in_maps.append(m)

    res = run_bass_kernel_spmd(nc, in_maps, core_ids=list(range(NCORES)))
    yT = np.concatenate([r["yT"] for r in res.results], axis=1)  # [D, N]
    return np.ascontiguousarray(yT.T)[None].astype(np.float32)

